# revision 18
# baseline (speedup 1.0000x reference)
"""Single-head causal attention kernel for Trainium2 (Bass/Tile), SPMD over 8 cores.

Problem: inputs [B=8, S=2048, E=1024]; Wq/Wk/Wv [E, H=1024]; bq/bk/bv [H].
  q = x@Wq+bq; k = x@Wk+bk; v = x@Wv+bv
  out = softmax(causal(q k^T / sqrt(H))) v        -> [B, S, H]

Sharding: data-parallel over batch, 1 batch element per NeuronCore (8 cores).

v2 design (all matmul operands bf16, fp32 PSUM accumulate, everything
SBUF-resident, single pass over x):
  - x streamed in 512-row chunks: fp32->bf16 cast (ScalarE), PE-transpose to
    xT[e,s] bf16 (resident for phases 1-2, freed before attention).
  - K pass, then Q pass (bias fused into the PSUM eviction as a per-partition
    activation bias), then V pass (bv folded into the final output eviction
    instead, since sum(attn)=1). Weights cycle through a 2-slot bf16 pool.
  - Attention per 512-wide q-chunk: scores^T[k,q] accumulated over 8 h-tiles,
    exp fused on ScalarE eviction (scale=1/32) -> attnT bf16; diagonal chunks
    masked with gpsimd.affine_select; O[q,h] accumulated over k-tiles with the
    softmax denominator Z as an extra N=1 ones-matmul in the same k loop
    (shares the attnT stationary); final eviction fuses *1/Z and +bv via
    vector.scalar_tensor_tensor.
"""

import numpy as np

import concourse.bass as bass
import concourse.bacc as bacc
import concourse.mybir as mybir
from concourse import tile
from concourse import bass_utils
from concourse.masks import make_identity

P = 128
F32 = mybir.dt.float32
BF16 = mybir.dt.bfloat16

B, S, E, H = 8, 2048, 1024, 1024
N_CORES = 8
QC = 256                       # q-chunk width in the attention phase
CW = 512                       # moving-operand chunk width (PSUM bank limit)


def attention_kernel(tc, out, x, wq, bq, wk, bk, wv, bv):
    nc = tc.nc
    ST, ET, HT = S // P, E // P, H // P      # 16, 8, 8 tiles
    NC = S // CW                             # 4 s-chunks
    NJ = S // QC                             # 4 q-chunks
    inv_sqrt_h = 1.0 / float(np.sqrt(H))

    from contextlib import ExitStack

    root = ExitStack()
    with root:
        # ---------------- constants ----------------
        const = root.enter_context(tc.tile_pool(name="const", bufs=1))
        ident = const.tile([P, P], F32, name="ident")
        make_identity(nc, ident)
        ones_col = const.tile([P, 1], BF16, name="ones_col")
        nc.gpsimd.memset(ones_col, 1.0)
        bk_sb = const.tile([P, HT], F32, name="bk_sb")
        nc.scalar.dma_start(bk_sb[:], bk.rearrange("(t p) -> p t", p=P))
        bq_sb = const.tile([P, HT], F32, name="bq_sb")
        nc.scalar.dma_start(bq_sb[:], bq.rearrange("(t p) -> p t", p=P))
        bv_row = const.tile([1, H], F32, name="bv_row")
        nc.scalar.dma_start(bv_row[:], bv.rearrange("(o h) -> o h", o=1))
        bv_bcast = const.tile([P, H], F32, name="bv_bcast")
        nc.gpsimd.partition_broadcast(bv_bcast[:], bv_row[:])
        # causal masks for diagonal tiles: keep where f - 128*m - p >= 0
        dmasks = []
        for m in range(QC // P):
            mk = const.tile([P, QC], BF16, name=f"dmask{m}")
            nc.gpsimd.memset(mk, 1.0)
            nc.gpsimd.affine_select(
                out=mk[:], in_=mk[:],
                compare_op=mybir.AluOpType.is_ge,
                fill=0.0, base=-m * P, channel_multiplier=-1,
                pattern=[[1, QC]])
            dmasks.append(mk)

        # ---------------- persistent bf16 arrays ----------------
        big = root.enter_context(tc.tile_pool(name="big", bufs=1))
        kT = big.tile([P, HT, S], BF16, name="kT")       # K^T  [h, s]
        qT = big.tile([P, HT, S], BF16, name="qT")       # Q^T  [h, s]
        v_sb = big.tile([P, ST, H], BF16, name="v_sb")   # V    [s, h]

        # ---------------- PSUM pools ----------------
        tps = root.enter_context(tc.tile_pool(name="tps", bufs=3, space="PSUM"))
        acc = root.enter_context(tc.tile_pool(name="acc", bufs=4, space="PSUM"))
        zps = root.enter_context(tc.tile_pool(name="zps", bufs=1, space="PSUM"))

        # ============== phases 1-2: projections (xT + weights scoped) ======
        with ExitStack() as ph:
            xt_pool = ph.enter_context(tc.tile_pool(name="xt", bufs=1))
            xT = xt_pool.tile([P, ET, S], BF16, name="xT")   # x^T [e, s]
            w_pool = ph.enter_context(tc.tile_pool(name="w", bufs=2))
            xf_pool = ph.enter_context(tc.tile_pool(name="xf", bufs=3))
            wf_pool = ph.enter_context(tc.tile_pool(name="wf", bufs=2))

            def load_weight(w_dram):
                w_bf = w_pool.tile([P, ET, H], BF16, name="w_bf")
                for e in range(ET):
                    wf = wf_pool.tile([P, H], F32, name="wf")
                    nc.scalar.dma_start(
                        wf[:], w_dram[e * P:(e + 1) * P, :])
                    nc.vector.tensor_copy(w_bf[:, e, :], wf[:])
                return w_bf

            F32R = mybir.dt.float32r

            def ingest_chunk(c):
                for ss in range(4):
                    si = 4 * c + ss
                    xf = xf_pool.tile([P, E], F32, name="xf")
                    nc.sync.dma_start(xf[:], x[si * P:(si + 1) * P, :])
                    for e in range(ET):
                        tp = tps.tile([P, P], F32, name="tp", space="PSUM")
                        nc.tensor.transpose(
                            tp[:], xf[:, e * P:(e + 1) * P], ident[:])
                        dst = xT[:, e, si * P:(si + 1) * P]
                        if (si * ET + e) % 2 == 0:
                            nc.vector.tensor_copy(dst, tp[:])
                        else:
                            nc.scalar.activation(
                                dst, tp[:],
                                mybir.ActivationFunctionType.Identity)

            # ---- PE warmup while the first x tiles are in flight ----
            warm = tps.tile([P, P], F32, name="tp", space="PSUM")
            for _ in range(8):
                nc.tensor.transpose(warm[:], ident[:], ident[:])

            # ---- K pass (x ingest runs one chunk ahead) ----
            ingest_chunk(0)
            wk_bf = load_weight(wk)
            for c in range(NC):
                if c + 1 < NC:
                    ingest_chunk(c + 1)
                for t in range(HT):
                    ap = acc.tile([P, CW], F32, name="mm", space="PSUM")
                    for e in range(ET):
                        nc.tensor.matmul(
                            ap[:],
                            wk_bf[:, e, t * P:(t + 1) * P],
                            xT[:, e, c * CW:(c + 1) * CW],
                            start=(e == 0), stop=(e == ET - 1))
                    if t % 2 == 0:
                        nc.scalar.activation(
                            kT[:, t, c * CW:(c + 1) * CW], ap[:],
                            mybir.ActivationFunctionType.Identity,
                            bias=bk_sb[:, t:t + 1])
                    else:
                        nc.vector.tensor_scalar_add(
                            kT[:, t, c * CW:(c + 1) * CW], ap[:],
                            bk_sb[:, t:t + 1])
                if c == 0:
                    wq_bf = load_weight(wq)

            # ---- Q pass ----
            for c in range(NC):
                for t in range(HT):
                    ap = acc.tile([P, CW], F32, name="mm", space="PSUM")
                    for e in range(ET):
                        nc.tensor.matmul(
                            ap[:],
                            wq_bf[:, e, t * P:(t + 1) * P],
                            xT[:, e, c * CW:(c + 1) * CW],
                            start=(e == 0), stop=(e == ET - 1))
                    if t % 2 == 0:
                        nc.scalar.activation(
                            qT[:, t, c * CW:(c + 1) * CW], ap[:],
                            mybir.ActivationFunctionType.Identity,
                            bias=bq_sb[:, t:t + 1])
                    else:
                        nc.vector.tensor_scalar_add(
                            qT[:, t, c * CW:(c + 1) * CW], ap[:],
                            bq_sb[:, t:t + 1])
                if c == 0:
                    wv_bf = load_weight(wv)

            # ---- V pass (no bias; folded into the final eviction) ----
            for i in range(ST):
                vps = [acc.tile([P, CW], F32, name="mm", space="PSUM")
                       for _ in range(2)]
                for e in range(ET):
                    for hc in range(2):
                        nc.tensor.matmul(
                            vps[hc][:],
                            xT[:, e, i * P:(i + 1) * P],
                            wv_bf[:, e, hc * CW:(hc + 1) * CW],
                            start=(e == 0), stop=(e == ET - 1))
                for hc in range(2):
                    if (i + hc) % 2 == 0:
                        nc.vector.tensor_copy(
                            v_sb[:, i, hc * CW:(hc + 1) * CW], vps[hc][:])
                    else:
                        nc.scalar.activation(
                            v_sb[:, i, hc * CW:(hc + 1) * CW], vps[hc][:],
                            mybir.ActivationFunctionType.Identity)

        # ================= phase 3: attention per q-chunk ==================
        at_pool = root.enter_context(tc.tile_pool(name="at", bufs=ST + 2))
        o_pool = root.enter_context(tc.tile_pool(name="o_st", bufs=3))
        rz_pool = root.enter_context(tc.tile_pool(name="rz", bufs=4))
        QS = QC // P                          # q-subtiles per chunk
        for j in range(NJ):
            nk = (j + 1) * QS                 # causal k-tiles for this chunk
            attnT = []
            for i in range(nk):
                sp = acc.tile([P, QC], F32, name="mm", space="PSUM")
                for t in range(HT):
                    nc.tensor.matmul(
                        sp[:],
                        kT[:, t, i * P:(i + 1) * P],
                        qT[:, t, j * QC:(j + 1) * QC],
                        start=(t == 0), stop=(t == HT - 1))
                at = at_pool.tile([P, QC], BF16, name="at")
                nc.scalar.activation(at[:], sp[:],
                                     mybir.ActivationFunctionType.Exp,
                                     scale=inv_sqrt_h)
                if i >= j * QS:               # tile touches the diagonal
                    nc.vector.tensor_mul(at[:], at[:],
                                         dmasks[i - j * QS][:])
                attnT.append(at)
            for qs in range(QS):
                q_glob = j * QS + qs
                zp = zps.tile([P, 1], F32, name="zp", space="PSUM")
                rz = rz_pool.tile([P, 1], F32, name="rz")
                o_stage = o_pool.tile([P, H], F32, name="o_stage")
                n_i = q_glob + 1
                tail = (j == NJ - 1 and qs == QS - 1)
                if tail:
                    # split per hc so hc0 evicts + DMAs while hc1 accumulates
                    for hc in range(2):
                        op = acc.tile([P, CW], F32, name="mm", space="PSUM")
                        for i in range(n_i):
                            a_slice = attnT[i][:, qs * P:(qs + 1) * P]
                            nc.tensor.matmul(
                                op[:], a_slice,
                                v_sb[:, i, hc * CW:(hc + 1) * CW],
                                start=(i == 0), stop=(i == n_i - 1))
                            if hc == 0:
                                nc.tensor.matmul(
                                    zp[:], a_slice, ones_col[:, :],
                                    start=(i == 0), stop=(i == n_i - 1))
                        if hc == 0:
                            nc.vector.reciprocal(rz[:], zp[:])
                        nc.vector.scalar_tensor_tensor(
                            out=o_stage[:, hc * CW:(hc + 1) * CW],
                            in0=op[:],
                            scalar=rz[:, 0:1],
                            in1=bv_bcast[:, hc * CW:(hc + 1) * CW],
                            op0=mybir.AluOpType.mult,
                            op1=mybir.AluOpType.add)
                        nc.sync.dma_start(
                            out[q_glob * P:(q_glob + 1) * P,
                                hc * CW:(hc + 1) * CW],
                            o_stage[:, hc * CW:(hc + 1) * CW])
                    continue
                ops = [acc.tile([P, CW], F32, name="mm", space="PSUM")
                       for _ in range(2)]
                for i in range(n_i):
                    a_slice = attnT[i][:, qs * P:(qs + 1) * P]
                    for hc in range(2):
                        nc.tensor.matmul(
                            ops[hc][:],
                            a_slice,
                            v_sb[:, i, hc * CW:(hc + 1) * CW],
                            start=(i == 0), stop=(i == n_i - 1))
                    nc.tensor.matmul(
                        zp[:], a_slice, ones_col[:, :],
                        start=(i == 0), stop=(i == n_i - 1))
                nc.vector.reciprocal(rz[:], zp[:])
                for hc in range(2):
                    nc.vector.scalar_tensor_tensor(
                        out=o_stage[:, hc * CW:(hc + 1) * CW],
                        in0=ops[hc][:],
                        scalar=rz[:, 0:1],
                        in1=bv_bcast[:, hc * CW:(hc + 1) * CW],
                        op0=mybir.AluOpType.mult,
                        op1=mybir.AluOpType.add)
                    nc.sync.dma_start(
                        out[q_glob * P:(q_glob + 1) * P,
                            hc * CW:(hc + 1) * CW],
                        o_stage[:, hc * CW:(hc + 1) * CW])


def build_program(n_cores=N_CORES):
    nc = bacc.Bacc("TRN2", target_bir_lowering=False, debug=False,
                   num_devices=n_cores)
    x = nc.dram_tensor("x", [S, E], F32, kind="ExternalInput").ap()
    wq = nc.dram_tensor("wq", [E, H], F32, kind="ExternalInput").ap()
    bq = nc.dram_tensor("bq", [H], F32, kind="ExternalInput").ap()
    wk = nc.dram_tensor("wk", [E, H], F32, kind="ExternalInput").ap()
    bk = nc.dram_tensor("bk", [H], F32, kind="ExternalInput").ap()
    wv = nc.dram_tensor("wv", [E, H], F32, kind="ExternalInput").ap()
    bv = nc.dram_tensor("bv", [H], F32, kind="ExternalInput").ap()
    out = nc.dram_tensor("out", [S, H], F32, kind="ExternalOutput").ap()
    with tile.TileContext(nc) as tc:
        attention_kernel(tc, out, x, wq, bq, wk, bk, wv, bv)
    nc.compile()
    return nc


def kernel(inputs, Wq, bq, Wk, bk, Wv, bv, _trace=False, _tmpdir=None):
    inputs = np.ascontiguousarray(inputs, dtype=np.float32)
    nc = build_program()
    in_maps = []
    for c in range(N_CORES):
        in_maps.append({
            "x": np.ascontiguousarray(inputs[c]),
            "wq": np.ascontiguousarray(Wq, dtype=np.float32),
            "bq": np.ascontiguousarray(bq, dtype=np.float32),
            "wk": np.ascontiguousarray(Wk, dtype=np.float32),
            "bk": np.ascontiguousarray(bk, dtype=np.float32),
            "wv": np.ascontiguousarray(Wv, dtype=np.float32),
            "bv": np.ascontiguousarray(bv, dtype=np.float32),
        })
    res = bass_utils.run_bass_kernel_spmd(
        nc, in_maps, core_ids=list(range(N_CORES)),
        trace=_trace, tmpdir=_tmpdir)
    out = np.stack([res.results[c]["out"] for c in range(N_CORES)], axis=0)
    if _trace:
        kernel.last_results = res
    return out
